# revision 1
# baseline (speedup 1.0000x reference)
# Multi-head attention (N=4, L=2048, D=1024, H=16, DK=64) on 8 NeuronCores.
#
# Sharding: pure data-parallel over (batch n, q-half) -> 8 shards, no
# collectives. Each core gets Q rows [n, qh*1024:(qh+1)*1024], full K/V of its
# batch, and the matching mask rows.
#
# Per-core pipeline (all layouts chosen so the contraction dim sits on SBUF
# partitions and softmax row-sums come out of the PE for free):
#   QiT[e,q] = wq^T-contract(qT)          (fp32r matmuls, bf16 result)
#   KiT[e,k] = wk^T-contract(kT)
#   Vi[k,e]  = vT-contract(wv), plus a ones column per head (row-sum trick)
#   per head h, per k-tile: S^T[k,q] = KiT_h^T-free matmul QiT_h  (contract 64)
#     P^T = exp(S^T/8) * maskT           (no max-subtraction needed: |S|/8 <~ 20)
#     pv[e_h|rowsum, q] += Vi_aug_h^T-contract P^T
#   headiT[e,q] = pv[0:64] / pv[64]      (flash-style deferred normalization,
#                                         1/r via exp(-ln r) on ScalarE)
#   out[q,d] = headiT^T-contract wo + bias
import sys

sys.path.insert(0, "/opt/trn_rl_repo")

from contextlib import ExitStack

import numpy as np
import ml_dtypes

N, QLEN, KLEN, DMODEL, NHEAD, DK = 4, 2048, 2048, 1024, 16, 64
NCORES = 8
P = 128
QS = N * QLEN // NCORES  # 1024 q rows per core
E = NHEAD * DK  # 1024
KO = KLEN // P  # 16 k-tiles
EO = E // P  # 8 e-tiles
DO = DMODEL // P  # 8 d-tiles

_prog_cache = {}


def _build_program():
    import concourse.tile as tile
    from concourse import bacc, mybir

    f32 = mybir.dt.float32
    f32r = mybir.dt.float32r
    bf16 = mybir.dt.bfloat16
    Exp = mybir.ActivationFunctionType.Exp
    Ln = mybir.ActivationFunctionType.Ln

    nc = bacc.Bacc("TRN2", target_bir_lowering=False, debug=False)

    qT = nc.dram_tensor("qT", (DMODEL, QS), f32r, kind="ExternalInput").ap()
    kT = nc.dram_tensor("kT", (DMODEL, KLEN), f32r, kind="ExternalInput").ap()
    vT = nc.dram_tensor("vT", (DMODEL, KLEN), f32r, kind="ExternalInput").ap()
    maskT = nc.dram_tensor("maskT", (KLEN, QS), bf16, kind="ExternalInput").ap()
    wq = nc.dram_tensor("wq", (DMODEL, E), f32r, kind="ExternalInput").ap()
    wk = nc.dram_tensor("wk", (DMODEL, E), f32r, kind="ExternalInput").ap()
    wv = nc.dram_tensor("wv", (DMODEL, E), f32r, kind="ExternalInput").ap()
    wo = nc.dram_tensor("wo", (E, DMODEL), bf16, kind="ExternalInput").ap()
    wob = nc.dram_tensor("wob", (1, DMODEL), bf16, kind="ExternalInput").ap()
    out = nc.dram_tensor("out", (QS, DMODEL), f32, kind="ExternalOutput").ap()

    qT_r = qT.rearrange("(do p) q -> p do q", p=P)
    kT_r = kT.rearrange("(do p) k -> p do k", p=P)
    vT_r = vT.rearrange("(do p) k -> p do k", p=P)
    wq_r = wq.rearrange("(do p) e -> p do e", p=P)
    wk_r = wk.rearrange("(do p) e -> p do e", p=P)
    wv_r = wv.rearrange("(do p) e -> p do e", p=P)
    wo_r = wo.rearrange("(eo p) d -> p eo d", p=P)
    maskT_r = maskT.rearrange("(ko p) q -> p ko q", p=P)

    with tile.TileContext(nc) as tc, ExitStack() as top:
        res = top.enter_context(tc.tile_pool(name="res", bufs=1))
        abc = ExitStack()
        gps = abc.enter_context(tc.tile_pool(name="gps", bufs=4, space="PSUM"))
        KiT_s = res.tile([P, EO, KLEN], bf16)  # e = eo*128+p
        QiT_s = res.tile([P, EO, QS], bf16)
        Vi_s = res.tile([P, KO, NHEAD * 65], bf16)  # k = ko*128+p; col h*65+64 = 1.0
        maskT_s = res.tile([P, KO, QS], bf16)
        headiT_s = res.tile([P, EO, QS], bf16)
        wo_s = res.tile([P, EO, DMODEL], bf16)
        wob_s = res.tile([P, DMODEL], bf16)

        # ---------- Phases A+B interleaved: KiT and QiT ----------
        # K is processed in two column halves, Q in two halves, alternating so
        # DMA of the next half overlaps compute of the current one.
        KH = KLEN // 2
        QH = QS // 2
        with ExitStack() as ph:
            xp = ph.enter_context(tc.tile_pool(name="xbuf", bufs=1))
            wpool = ph.enter_context(tc.tile_pool(name="wtile", bufs=2))
            for half in range(2):
                # --- K half ---
                kT_s = xp.tile([P, DO, KH], f32r, tag="kT_s", name=f"kT_s{half}")
                for cc in range(KH // 512):
                    for do in range(DO):
                        nc.sync.dma_start(
                            kT_s[:, do, cc * 512 : (cc + 1) * 512],
                            kT_r[:, do, half * KH + cc * 512 : half * KH + (cc + 1) * 512],
                        )
                for eo in range(EO):
                    wcol = wpool.tile(
                        [P, DO, P], f32r, tag="wcol", name=f"wk{half}_{eo}"
                    )
                    nc.sync.dma_start(wcol[:], wk_r[:, :, eo * P : (eo + 1) * P])
                    for c in range(KH // 512):
                        pt = gps.tile([P, 512], f32, tag="gps", name=f"psk{half}_{eo}_{c}")
                        for do in range(DO):
                            nc.tensor.matmul(
                                pt[:],
                                lhsT=wcol[:, do],
                                rhs=kT_s[:, do, c * 512 : (c + 1) * 512],
                                start=(do == 0),
                                stop=(do == DO - 1),
                            )
                        eng = nc.vector if (eo + c) % 2 == 0 else nc.scalar
                        if eng is nc.vector:
                            nc.vector.tensor_copy(
                                out=KiT_s[
                                    :, eo,
                                    half * KH + c * 512 : half * KH + (c + 1) * 512,
                                ],
                                in_=pt[:],
                            )
                        else:
                            nc.scalar.copy(
                                out=KiT_s[
                                    :, eo,
                                    half * KH + c * 512 : half * KH + (c + 1) * 512,
                                ],
                                in_=pt[:],
                            )
                # --- Q half ---
                qT_s = xp.tile([P, DO, QH], f32r, tag="qT_s", name=f"qT_s{half}")
                for do in range(DO):
                    nc.sync.dma_start(
                        qT_s[:, do], qT_r[:, do, half * QH : (half + 1) * QH]
                    )
                for eo in range(EO):
                    wcol = wpool.tile(
                        [P, DO, P], f32r, tag="wcol", name=f"wq{half}_{eo}"
                    )
                    nc.sync.dma_start(wcol[:], wq_r[:, :, eo * P : (eo + 1) * P])
                    pt = gps.tile([P, 512], f32, tag="gps", name=f"psq{half}_{eo}")
                    for do in range(DO):
                        nc.tensor.matmul(
                            pt[:],
                            lhsT=wcol[:, do],
                            rhs=qT_s[:, do],
                            start=(do == 0),
                            stop=(do == DO - 1),
                        )
                    nc.vector.tensor_copy(
                        out=QiT_s[:, eo, half * QH : (half + 1) * QH], in_=pt[:]
                    )

        # ---------- Phase C: Vi = V @ WV (k-major) + ones columns ----------
        with ExitStack() as ph:
            wres = ph.enter_context(tc.tile_pool(name="wvres", bufs=1))
            vtp = ph.enter_context(tc.tile_pool(name="vttile", bufs=3))
            wv_s = wres.tile([P, DO, E], f32r)
            for cc in range(2):
                for do in range(DO):
                    nc.sync.dma_start(
                        wv_s[:, do, cc * 512 : (cc + 1) * 512],
                        wv_r[:, do, cc * 512 : (cc + 1) * 512],
                    )
            nc.vector.memset(Vi_s[:, :, 64::65], 1.0)  # the ones columns
            for ko in range(KO):
                vcol = vtp.tile([P, DO, P], f32r, tag="vcol", name=f"vcol{ko}")
                nc.sync.dma_start(vcol[:], vT_r[:, :, ko * P : (ko + 1) * P])
                for c in range(E // 512):
                    pt = gps.tile([P, 512], f32, tag="gps", name=f"psv{ko}_{c}")
                    for do in range(DO):
                        nc.tensor.matmul(
                            pt[:],
                            lhsT=vcol[:, do],
                            rhs=wv_s[:, do, c * 512 : (c + 1) * 512],
                            start=(do == 0),
                            stop=(do == DO - 1),
                        )
                    dst = Vi_s[:, ko, :].rearrange("p (h j) -> p h j", j=65)[
                        :, c * 8 : (c + 1) * 8, 0:64
                    ]
                    if (ko + c) % 2 == 0:
                        nc.vector.tensor_copy(
                            out=dst, in_=pt[:].rearrange("p (h j) -> p h j", j=64)
                        )
                    else:
                        nc.scalar.copy(
                            out=dst, in_=pt[:].rearrange("p (h j) -> p h j", j=64)
                        )

        # loads needed by phases D/E
        for ko in range(KO):
            nc.sync.dma_start(maskT_s[:, ko], maskT_r[:, ko])
        for eo in range(EO):
            nc.sync.dma_start(wo_s[:, eo], wo_r[:, eo])
        nc.sync.dma_start(wob_s[:, None, :], wob.partition_broadcast(P))

        abc.close()

        # ---------- Phase D: attention, two heads interleaved per pair ----------
        # Heads 2hp (partitions 0-63) and 2hp+1 (64-127): their K=64 S^T
        # matmuls use different PE row groups. PV for k-tile ko-2 is emitted
        # after S^T/exp/mask of ko (skew 2) so PE never waits on the
        # exp+mask chain.
        rs_all = res.tile([NHEAD, QS], f32)
        SKEW = 2
        with ExitStack() as ph:
            sps = ph.enter_context(tc.tile_pool(name="spsum", bufs=2, space="PSUM"))
            pvs = ph.enter_context(tc.tile_pool(name="pvsum", bufs=1, space="PSUM"))
            pp = ph.enter_context(tc.tile_pool(name="ptile", bufs=SKEW + 4))
            for hp in range(NHEAD // 2):
                pv = [
                    pvs.tile([P, QS], f32, tag=f"pv{i}", name=f"pv{i}_{hp}")
                    for i in range(2)
                ]
                ptq = {}

                def emit_pv(ko, hp=hp, pv=pv, ptq=ptq):
                    for c in range(QS // 512):
                        pt = ptq.pop((ko, c))
                        for i in range(2):
                            h = 2 * hp + i
                            nc.tensor.matmul(
                                pv[i][0:65, c * 512 : (c + 1) * 512],
                                lhsT=Vi_s[:, ko, h * 65 : (h + 1) * 65],
                                rhs=pt[:, i * 512 : (i + 1) * 512],
                                start=(ko == 0),
                                stop=(ko == KO - 1),
                                skip_group_check=True,
                            )

                for ko in range(KO):
                    # one S^T psum tile per q-chunk holds BOTH heads side by
                    # side: the two K=64 matmuls hit different PE row groups
                    # back-to-back, and one exp covers the pair.
                    for c in range(QS // 512):
                        st = sps.tile(
                            [P, QS], f32, tag="st", name=f"st_{hp}_{ko}_{c}"
                        )
                        for i in range(2):
                            p0 = 64 * i
                            nc.tensor.matmul(
                                st[:, i * 512 : (i + 1) * 512],
                                lhsT=KiT_s[p0 : p0 + 64, hp, ko * P : (ko + 1) * P],
                                rhs=QiT_s[p0 : p0 + 64, hp, c * 512 : (c + 1) * 512],
                                start=True,
                                stop=True,
                            )
                        pt = pp.tile(
                            [P, QS], bf16, tag=f"pt{c}", name=f"pt{c}_{hp}_{ko}"
                        )
                        nc.scalar.activation(
                            out=pt[:], in_=st[:], func=Exp, scale=0.125
                        )
                        nc.vector.tensor_mul(
                            out=pt[:].rearrange("p (i q) -> p i q", i=2),
                            in0=pt[:].rearrange("p (i q) -> p i q", i=2),
                            in1=maskT_s[:, ko, None, c * 512 : (c + 1) * 512]
                            .to_broadcast([P, 2, 512]),
                        )
                        ptq[(ko, c)] = pt
                    if ko >= SKEW:
                        emit_pv(ko - SKEW)
                for ko in range(KO - SKEW, KO):
                    emit_pv(ko)
                # copy out unnormalized heads + row sums (normalized later)
                for i in range(2):
                    h = 2 * hp + i
                    nc.vector.tensor_copy(
                        out=headiT_s[64 * i : 64 * i + 64, hp, :], in_=pv[i][0:64, :]
                    )
                    rstmp = pp.tile([1, QS], f32, tag="rstmp", name=f"rstmp_{h}")
                    nc.vector.tensor_copy(out=rstmp[:], in_=pv[i][64:65, :])
                    nc.sync.dma_start(rs_all[h : h + 1, :], rstmp[:])

        # deferred normalization: 1/r = exp(-ln r) on ScalarE (packed 16 rows),
        # broadcast via two partition-broadcast DMAs from DRAM, one multiply.
        rs_dram = nc.dram_tensor("rs_scratch", (NHEAD, QS), f32).ap()
        with ExitStack() as ph:
            rp = ph.enter_context(tc.tile_pool(name="recip", bufs=1))
            rsinv = rp.tile([NHEAD, QS], f32, tag="rsinv")
            nc.scalar.activation(out=rsinv[:], in_=rs_all[:], func=Ln)
            nc.scalar.activation(out=rsinv[:], in_=rsinv[:], func=Exp, scale=-1.0)
            nc.sync.dma_start(rs_dram[:, :], rsinv[:])
            rrep_all = rp.tile([P, EO, QS], f32, tag="rrep_all")
            nc.sync.dma_start(
                rrep_all[0:64, :, :],
                rs_dram[0::2, :].partition_broadcast(64),
            )
            nc.sync.dma_start(
                rrep_all[64:128, :, :],
                rs_dram[1::2, :].partition_broadcast(64),
            )
            nc.vector.tensor_mul(out=headiT_s[:], in0=headiT_s[:], in1=rrep_all[:])

        # ---------- Phase E: out = headiT^T @ wo + bias ----------
        with ExitStack() as ph:
            pse = ph.enter_context(tc.tile_pool(name="psE", bufs=4, space="PSUM"))
            ot = ph.enter_context(tc.tile_pool(name="otile", bufs=3))
            for qt in range(QS // P):
                for c in range(DMODEL // 512):
                    pt = pse.tile([P, 512], f32, tag="psE", name=f"pso{qt}_{c}")
                    for eo in range(EO):
                        nc.tensor.matmul(
                            pt[:],
                            lhsT=headiT_s[:, eo, qt * P : (qt + 1) * P],
                            rhs=wo_s[:, eo, c * 512 : (c + 1) * 512],
                            start=(eo == 0),
                            stop=(eo == EO - 1),
                        )
                    o = ot.tile([P, 512], f32, tag="otile", name=f"o{qt}_{c}")
                    nc.vector.tensor_add(
                        out=o[:], in0=pt[:], in1=wob_s[:, c * 512 : (c + 1) * 512]
                    )
                    nc.sync.dma_start(
                        out[qt * P : (qt + 1) * P, c * 512 : (c + 1) * 512], o[:]
                    )

    nc.compile()
    return nc


def get_program():
    if "nc" not in _prog_cache:
        _prog_cache["nc"] = _build_program()
    return _prog_cache["nc"]


def make_in_maps(K, Q, V, mask, WQ, WK, WV, WO_w, WO_b):
    bf = ml_dtypes.bfloat16
    K = np.asarray(K, dtype=np.float32)
    Q = np.asarray(Q, dtype=np.float32)
    V = np.asarray(V, dtype=np.float32)
    mask = np.asarray(mask)
    # head-concat weights: (H, D, DK) -> (D, H*DK)
    wq_h = np.ascontiguousarray(
        np.asarray(WQ, dtype=np.float32).transpose(1, 0, 2).reshape(DMODEL, E)
    )
    wk_h = np.ascontiguousarray(
        np.asarray(WK, dtype=np.float32).transpose(1, 0, 2).reshape(DMODEL, E)
    )
    wv_h = np.ascontiguousarray(
        np.asarray(WV, dtype=np.float32).transpose(1, 0, 2).reshape(DMODEL, E)
    )
    wo_h = np.ascontiguousarray(np.asarray(WO_w, dtype=np.float32).T).astype(bf)
    wob_h = np.asarray(WO_b, dtype=np.float32).reshape(1, DMODEL).astype(bf)

    kT_b = [np.ascontiguousarray(K[n].T) for n in range(N)]
    vT_b = [np.ascontiguousarray(V[n].T) for n in range(N)]

    in_maps = []
    for c in range(NCORES):
        n, qh = c // 2, c % 2
        qs = slice(qh * QS, (qh + 1) * QS)
        in_maps.append(
            {
                "qT": np.ascontiguousarray(Q[n, qs, :].T),
                "kT": kT_b[n],
                "vT": vT_b[n],
                "maskT": np.ascontiguousarray(mask[n, 0, qs, :].T).astype(bf),
                "wq": wq_h,
                "wk": wk_h,
                "wv": wv_h,
                "wo": wo_h,
                "wob": wob_h,
            }
        )
    return in_maps


def kernel(K, Q, V, mask, WQ, WK, WV, WO_w, WO_b):
    from concourse import bass_utils

    nc = get_program()
    in_maps = make_in_maps(K, Q, V, mask, WQ, WK, WV, WO_w, WO_b)
    res = bass_utils.run_bass_kernel_spmd(
        nc, in_maps, core_ids=list(range(NCORES)), trace=False
    )
    out = np.empty((N, QLEN, DMODEL), dtype=np.float32)
    for c in range(NCORES):
        n, qh = c // 2, c % 2
        out[n, qh * QS : (qh + 1) * QS, :] = res.results[c]["out"]
    return out

